# revision 35
# baseline (speedup 1.0000x reference)
"""Trainium2 Bass kernel for nn_AttentionGuidedIterativeBlock.

Math reformulation: the (B,L,P,D) phasor cumsum + retrieval is causal linear
attention with feature map Kf = [cos(phases), sin(phases)] (2P=64 dims):

    retrieved[l] = (sum_{l'<=l} (Qf[l].Kf[l']) * V[l']) / (sqrt(l+1)*sqrt(P))

Sharding: 8 cores x 512 tokens (cores 0-3 batch 0, 4-7 batch 1).  Each core
rebuilds the prefix state S = Kf^T @ V_aug over its batch prefix from a
per-core slot-permuted copy of x (12 prefix slots, zero-padded + masked via a
per-slot km scalar, then the 4 own chunks), so the program is uniform across
cores while own-segment K/V prep and prefix-state accumulation share the same
matmuls.  Everything runs in bf16 on the PE (fp32 PSUM accumulate); LayerNorm
mean-subtraction is folded into w1/wog as a rank-1 matmul (-mr x colsum(W)),
gelu(exact erf) is replaced by the sigmoid approximation expressed as
silu(1.702x)/1.702 so every activation function used stays inside one
activation-table set except nothing -- zero table swaps except none.
"""

import math
import os

import numpy as np
import ml_dtypes

D, P, I, H = 256, 32, 3, 8
B, L = 2, 2048
NCORES = 8
SEG = 512          # tokens per core
CH = 128           # chunk (tile partition) size
NSLOT = 16         # 12 prefix slots + 4 own chunks
NPREF = 12
HSEG = 256
PI = math.pi
EPS = 1e-5
GSC = 1.702        # sigmoid-gelu: gelu(x) ~= silu(GSC*x)/GSC

_CACHE = {}

# Source-content salt folded into a tensor name: any kernel.py change makes a
# distinct NEFF signature, so the neuron compile cache can never serve a NEFF
# built from a different version of this file.
import hashlib
with open(__file__, "rb") as _f:
    _SALT = hashlib.md5(_f.read()).hexdigest()[:8]

# ---- shared-blob column offsets (bf16, 128 partitions) ----
# blobA (early consts): tvpe (2,288), ident(128), tril(128), oc264, oc256,
#   onescol, onesrow(row0,128), pe_b_row(row0,32)
A_TVPE = 0
A_ID = 576
A_TRIL = 704
A_OC264 = 832
A_OC256 = 833
A_ONEC = 834
A_ONER = 835          # row0 cols 835:1347 = 1.0
A_PEBR = 1347         # row0 cols 1347:1379
A_MQBH = 1379         # row0 cols 1379:1387 = 0.5*mq_b
A_COLS = 1387

# blobB (weights): pe_w (2,32), mq_w (2,8), wog (2,2,128), w1k I*(2,512),
#   w2k I*(4,2,128), gwk 2*(4,2,128), w1k2 I*(8rows,512), w1sum_neg I*(row0,512),
#   wogsum_neg (row0, 256)
B_PEW = 0
B_MQW = 64
B_WOG = 80
B_W1K = 592
B_W2K = B_W1K + I * 1024          # 3664
B_GWK = B_W2K + I * 1024          # 6736
B_W1K2 = B_GWK + 2 * 1024         # 8784 (rows 0:8)
B_W1SUM = B_W1K2 + I * 512        # 10320 (row0)
B_WOGSUM = B_W1SUM + I * 512      # 11856 (row0)
B_COLS = B_WOGSUM + 256           # 12112

# blobF (fp32 biases): pe_b_col(32,1), mq_bh_col(8,1), b1s I*(128,4),
#   b2 I*(128,2), gb 2*(128,2)
F_PEB = 0
F_MQB = 1
F_B1 = 2
F_B2 = F_B1 + I * 4       # 14
F_GB = F_B2 + I * 2       # 20
F_TRIL = F_GB + 4         # 24
F_COLS = F_TRIL + 128     # 152

# blobC (per-core): x_perm (2,2048), invn (rows0:64, 512), km (16)
C_X = 0
C_INVN = 4096
C_KM = 4608
C_COLS = 4624


def _patch_walrus_passes():
    import concourse.bass_utils as bu
    if getattr(bu, "_nv_patched", False):
        return
    orig = bu.run_command

    def patched(cmd, cwd=None, **kw):
        cmd = list(cmd)
        if "--pass" in cmd:
            i = cmd.index("--pass")
            cmd[i + 1] = cmd[i + 1].replace("birverifier,", "")
        return orig(cmd, cwd=cwd, **kw)

    bu.run_command = patched
    bu._nv_patched = True


def _build_program(split=True):
    _patch_walrus_passes()
    import concourse.bass as bass
    import concourse.tile as tile
    from concourse import mybir

    f32 = mybir.dt.float32

    nc = bass.Bass("TRN2", target_bir_lowering=False, debug=False,
                   num_devices=NCORES)

    def din(name, shape, dt):
        return nc.dram_tensor(name, shape, dt, kind="ExternalInput").ap()

    bf16 = mybir.dt.bfloat16
    t = {}
    t["blobA_" + _SALT] = din("blobA_" + _SALT, (CH, A_COLS), bf16)
    t["blobA"] = t["blobA_" + _SALT]
    t["blobB"] = din("blobB", (CH, B_COLS), bf16)
    t["blobC"] = din("blobC", (CH, C_COLS), bf16)
    t["blobF"] = din("blobF", (CH, F_COLS), f32)
    t["tvb"] = din("tvb", (1, D), f32)
    t["x_fm"] = din("x_fm", (CH, 2, SEG), f32)
    t["y"] = nc.dram_tensor("y", (CH, 2, SEG), f32, kind="ExternalOutput").ap()
    if os.environ.get("DEBUG_DUMP"):
        def dout(name, shape):
            t[name] = nc.dram_tensor(name, shape, f32,
                                     kind="ExternalOutput").ap()
        dout("d_qfs", (2 * P, SEG))
        dout("d_at", (H, SEG))
        dout("d_rt", (CH, 2, SEG))
        dout("d_cn", (CH, 2, SEG))
        dout("d_rstd", (2, SEG))
        dout("d_hh", (CH, 4, SEG))
        dout("d_rf", (CH, 2, SEG))
        dout("d_q1", (CH, 2, SEG))
        dout("d_sh", (2 * P, 2, D))

    with tile.TileContext(nc) as tc:
        _body(tc, nc, t, f32, bf16, bass, mybir)
    if split:
        _split_waits(nc, mybir)
    return nc


def _split_waits(nc, mybir, cap=1):
    """Move excess sync-waits onto preceding same-engine NOPs (this walrus
    build allows one wait slot per instruction)."""
    for fn in nc.m.functions:
        for blk in fn.blocks:
            out = []
            for ins in blk.instructions:
                si = ins.sync_info
                if si is not None and len(si.on_wait) > cap:
                    waits = list(si.on_wait)
                    extra, keep = waits[:-cap], waits[-cap:]
                    for j, w in enumerate(extra):
                        nop = mybir.InstNoOp(name=f"{ins.name}_wsplit{j}",
                                             ins=[], outs=[])
                        nop.engine = ins.engine
                        nop.sync_info = mybir.SyncInfo(on_wait=[w],
                                                       on_update=[])
                        out.append(nop)
                    ins.sync_info = mybir.SyncInfo(on_wait=keep,
                                                   on_update=si.on_update)
                out.append(ins)
            blk.instructions = out


def _body(tc, nc, t, f32, bf16, bass, mybir):
    from concourse.alu_op_type import AluOpType as OP

    AF = mybir.ActivationFunctionType
    AX = mybir.AxisListType.X

    lp = nc.allow_low_precision(reason="bf16 kernel; tolerance 2e-2 validated")
    lp.__enter__()
    consts = tc.alloc_tile_pool(name="consts", bufs=1)
    own = tc.alloc_tile_pool(name="own", bufs=1)
    pa = tc.alloc_tile_pool(name="pa", bufs=2)
    pb = tc.alloc_tile_pool(name="pb", bufs=2)

    dma = nc.sync.dma_start
    mm = nc.tensor.matmul

    # ---- constant blobs (6 input DMAs total) ----
    cC = consts.tile([CH, C_COLS], bf16)
    dma(out=cC, in_=t["blobC"])
    cA = consts.tile([CH, A_COLS], bf16)
    dma(out=cA, in_=t["blobA"])
    cB = consts.tile([CH, B_COLS], bf16)
    dma(out=cB, in_=t["blobB"])
    cF = consts.tile([CH, F_COLS], f32)
    dma(out=cF, in_=t["blobF"])
    tvb_bc = consts.tile([2 * P, D], f32)
    dma(out=tvb_bc, in_=t["tvb"].to_broadcast((2 * P, D)))
    xfm_sb = consts.tile([CH, 2, SEG], f32)
    dma(out=xfm_sb, in_=t["x_fm"])

    def xslot(kh, s):
        return cC[:, C_X + kh * 2048 + s * CH: C_X + kh * 2048 + (s + 1) * CH]

    tvpe = lambda kh: cA[:, A_TVPE + kh * 288: A_TVPE + (kh + 1) * 288]
    ident = cA[:, A_ID:A_ID + CH]
    tril = cF[:, F_TRIL:F_TRIL + CH]
    oc264 = cA[:, A_OC264:A_OC264 + 1]
    oc256 = cA[:, A_OC256:A_OC256 + 1]
    onesr = cA[0:1, A_ONER:A_ONER + CH]
    onesr512 = cA[0:1, A_ONER:A_ONER + SEG]
    mqbh_row = cA[0:1, A_MQBH:A_MQBH + H]
    pebr = cA[0:1, A_PEBR:A_PEBR + P]
    invn = cC[0:2 * P, C_INVN:C_INVN + SEG]
    pe_b_col = cF[0:P, F_PEB:F_PEB + 1]
    mq_bh_col = cF[0:H, F_MQB:F_MQB + 1]
    halfpi = consts.tile([CH, 1], f32)
    nc.vector.memset(halfpi, PI / 2)
    i32 = mybir.dt.int32
    qmagic = consts.tile([1, SEG], i32)
    nc.vector.memset(qmagic, 0x5F3759DF)
    ones8f = consts.tile([H, 1], f32)
    nc.vector.memset(ones8f, 1.0)
    ones1x8f = consts.tile([1, H], f32)
    nc.vector.memset(ones1x8f, 1.0)
    eps_row = consts.tile([1, SEG], bf16)
    nc.vector.memset(eps_row, EPS)
    c264 = consts.tile([1, 1], f32)
    nc.vector.memset(c264, 1.0 / (D + H))
    f32r = mybir.dt.float32r
    rr = lambda ap: ap.bitcast(f32r)

    def ln_rstd(st2_psum, msq, tagp, ln=SEG):
        # rstd = 1/sqrt(st2 - m^2), DVE-only (Quake seed + 1 Newton);
        # eps is pre-accumulated into st2 via a K=1 matmul.
        var = pb.tile([1, ln], f32, tag=tagp + "var")
        nc.vector.tensor_tensor(var, st2_psum, msq, OP.subtract)
        sh = pb.tile([1, ln], i32, tag=tagp + "sh")
        nc.vector.tensor_scalar(sh, var.bitcast(i32), 1, None,
                                OP.logical_shift_right)
        si = pb.tile([1, ln], i32, tag=tagp + "si")
        nc.vector.tensor_tensor(si, qmagic[:, 0:ln], sh, OP.subtract)
        y = si.bitcast(f32)
        y2 = pb.tile([1, ln], f32, tag=tagp + "y2")
        nc.vector.tensor_tensor(y2, y, y, OP.mult)
        tn = pb.tile([1, ln], f32, tag=tagp + "tn")
        nc.vector.tensor_tensor(tn, y2, var, OP.mult)
        un = pb.tile([1, ln], f32, tag=tagp + "un")
        nc.vector.tensor_scalar(un, tn, -0.5, 1.5, OP.mult, OP.add)
        rstd = pb.tile([1, ln], bf16, tag=tagp + "rstd")
        nc.vector.tensor_tensor(rstd, y, un, OP.mult)
        return rstd

    pe_w = lambda kh: cB[:, B_PEW + kh * P: B_PEW + (kh + 1) * P]
    mq_w = lambda kh: cB[:, B_MQW + kh * H: B_MQW + (kh + 1) * H]
    wog = lambda kh, mh: cB[:, B_WOG + (kh * 2 + mh) * CH:
                            B_WOG + (kh * 2 + mh + 1) * CH]
    w1k = lambda it, kh, o: cB[:, B_W1K + it * 1024 + kh * 512 + o * CH:
                               B_W1K + it * 1024 + kh * 512 + (o + 1) * CH]
    w2k = lambda it, kh, mh: cB[:, B_W2K + it * 1024 + (kh * 2 + mh) * CH:
                                B_W2K + it * 1024 + (kh * 2 + mh + 1) * CH]
    gwk = lambda it, kh, mh: cB[:, B_GWK + it * 1024 + (kh * 2 + mh) * CH:
                                B_GWK + it * 1024 + (kh * 2 + mh + 1) * CH]
    w1k29 = lambda it, o: cB[0:33, B_W1K2 + it * 512 + o * CH:
                             B_W1K2 + it * 512 + (o + 1) * CH]
    wogsum = lambda mh: cB[0:1, B_WOGSUM + mh * CH:B_WOGSUM + (mh + 1) * CH]

    # =========== phase A: prefix state + own K/V (unified slot loop) =======
    psA = tc.alloc_tile_pool(name="psA", bufs=1, space="PSUM")


    S_ps = psA.tile([2 * P, D + 8], f32, tag="S")
    own_kf = own.tile([CH, 4, 2 * P], bf16)    # own kf, token-major, unmasked
    v_own = own.tile([CH, 4, D + 8], bf16)     # own V_aug, token-major

    for g in range(8):
        # 2-slot groups with double-buffered PSUM keep the PE gap-free
        vq_ps = psA.tile([CH, 2, 512], f32, tag="vq", bufs=2,
                         name=f"vq{g}")
        for c in range(2):
            s = 2 * g + c
            mm(vq_ps[:, c, 0:D + P], xslot(0, s), tvpe(0),
               start=True, stop=False, skip_group_check=True)
            mm(vq_ps[:, c, 0:D + P], xslot(1, s), tvpe(1),
               start=False, stop=False, skip_group_check=True)
            mm(vq_ps[:, c, D:D + P], onesr, pebr, start=False, stop=True,
               skip_group_check=True)
        tq = pa.tile([CH, 2, P], bf16, tag="tq")
        nc.scalar.activation(tq, vq_ps[:, :, D:D + P], AF.Tanh)
        aq = pa.tile([CH, 2, P], bf16, tag="aq")
        nc.scalar.activation(aq, tq, AF.Abs)
        j = 2 * (g - 6)
        kfg = (own_kf[:, j:j + 2, :] if g >= 6
               else pa.tile([CH, 2, 2 * P], bf16, tag="kf"))
        nc.scalar.activation(kfg[:, :, 0:P], aq, AF.Sin, scale=-PI,
                             bias=halfpi)
        nc.scalar.activation(kfg[:, :, P:2 * P], tq, AF.Sin, scale=PI)
        vdst = (v_own[:, j:j + 2, :] if g >= 6
                else pa.tile([CH, 2, D + 8], bf16, tag="vsb"))
        nc.vector.tensor_copy(vdst[:, :, 0:D], vq_ps[:, :, 0:D])
        nc.vector.memset(vdst[:, :, D:D + 8], 1.0)
        if g < 6:
            kfm = pa.tile([CH, 2, 2 * P], bf16, tag="kfm")
            nc.vector.tensor_tensor(
                kfm, kfg,
                cC[:, C_KM + 2 * g:C_KM + 2 * g + 2].unsqueeze(-1)
                .broadcast_to([CH, 2, 2 * P]), OP.mult)
            for c in range(2):
                s = 2 * g + c
                mm(S_ps, kfm[:, c, :], vdst[:, c, :],
                   start=(s == 0), stop=(s == NPREF - 1),
                   skip_group_check=True)

    # S_h0 = prefix state (+ rank-1 tv_b fold);  S_h1 adds own chunks 0,1
    S_h = []
    for hi in range(2):
        if hi == 1:
            for c in range(2):
                mm(S_ps, own_kf[:, c, :], v_own[:, c, :],
                   start=False, stop=(c == 1), skip_group_check=True)
        kfsum = own.tile([2 * P, 1], f32, tag=f"kfsum{hi}")
        nc.vector.tensor_copy(kfsum, S_ps[:, D:D + 1])
        st = own.tile([2 * P, D], f32, tag=f"S_tmp{hi}")
        nc.vector.tensor_tensor(
            st, tvb_bc, kfsum.broadcast_to([2 * P, D]), OP.mult)
        sh = own.tile([2 * P, D], bf16, tag=f"S_h{hi}")
        nc.vector.tensor_tensor(sh, st, S_ps[:, 0:D], OP.add)
        S_h.append(sh)

    # kff: own kf feature-major (for intra scores + iteration-0 Qf)
    kff = own.tile([2 * P, SEG], bf16)
    for c in range(4):
        tr_ps = psA.tile([2 * P, CH], bf16, tag="tr", bufs=2, name="tr_ps")
        nc.tensor.transpose(tr_ps, own_kf[:, c, :], ident)
        nc.vector.tensor_copy(kff[:, c * CH:(c + 1) * CH], tr_ps)

    acc_l = []
    for h in range(2):
        acc_h = own.tile([CH, 2, HSEG], bf16, tag=f"acc{h}",
                         name=f"acc{h}")
        nc.vector.memset(acc_h, 0.0)
        acc_l.append(acc_h)
    qA = own.tile([CH, 2, SEG], bf16)
    qB = own.tile([CH, 2, SEG], bf16)
    # initial query = own x (bf16, feature-major) = slots 12-15 of x_perm
    for kh in range(2):
        nc.vector.tensor_copy(
            qA[:, kh, :],
            cC[:, C_X + kh * 2048 + NPREF * CH:C_X + kh * 2048 + NSLOT * CH])

    psA.release()
    psB = tc.alloc_tile_pool(name="psB", bufs=1, space="PSUM")

    # intra score blocks per half: (key chunk, local query lo, n)
    HALF_BLOCKS = {0: [(0, 0, 2 * CH), (1, CH, CH)],
                   1: [(2, 0, 2 * CH), (3, CH, CH)]}

    # =========== refinement iterations (full-width, feature-major) ========
    for it in range(I):
        q = qA if it % 2 == 0 else qB
        qn = qB if it % 2 == 0 else qA

        # Qf (feature-major), per token-half so h0's retrieval overlaps
        # h1's activation chain; 1/norm folded
        qfs_l = []
        for qh in range(2):
            qsl = slice(qh * HSEG, (qh + 1) * HSEG)
            if it > 0:
                if qh == 0:
                    qp_ps = psB.tile([P, SEG], f32, tag="mix", bufs=2,
                                     name="qp_ps")
                    mm(qp_ps, pe_w(0), q[:, 0, :], start=True, stop=False)
                    mm(qp_ps, pe_w(1), q[:, 1, :], start=False, stop=True)
                tq_ = pb.tile([P, HSEG], bf16, tag="tq")
                nc.scalar.activation(tq_, qp_ps[:, qsl], AF.Tanh,
                                     bias=pe_b_col)
                aq_ = pb.tile([P, HSEG], bf16, tag="aq")
                nc.scalar.activation(aq_, tq_, AF.Abs)
                qf_h = pb.tile([2 * P, HSEG], bf16, tag="qf")
                nc.scalar.activation(qf_h[0:P, :], aq_, AF.Sin, scale=-PI,
                                     bias=halfpi[0:P, :])
                nc.scalar.activation(qf_h[P:2 * P, :], tq_, AF.Sin, scale=PI)
            else:
                qf_h = kff[:, qsl]
            qfs_h = pb.tile([2 * P, HSEG], bf16, tag=f"qfs{qh}")
            nc.vector.tensor_tensor(qfs_h, qf_h, invn[:, qsl], OP.mult)
            qfs_l.append(qfs_h)
        if os.environ.get("DEBUG_DUMP") and it == 0:
            dq = consts.tile([2 * P, SEG], f32, tag="dbg_q")
            nc.vector.tensor_copy(dq, qfs)
            dma(out=t["d_qfs"], in_=dq)
            ds0 = consts.tile([2 * P, 2, D], f32, tag="dbg_s")
            nc.vector.tensor_copy(ds0[:, 0, :], S_h[0])
            nc.vector.tensor_copy(ds0[:, 1, :], S_h[1])
            dma(out=t["d_sh"], in_=ds0)

        # attention: feature-major logits, token-major tanh-softmax
        # (keeps reciprocals at 8/32 elems per DVE lane), mq_b/2 folded in
        # via a K=1 rank-1 matmul
        z_ps = psB.tile([H, SEG], f32, tag="mix", bufs=2, name="z_ps")
        mm(z_ps, mq_w(0), q[:, 0, :], start=True, stop=False,
           skip_group_check=True)
        mm(z_ps, mq_w(1), q[:, 1, :], start=False, stop=False,
           skip_group_check=True)
        mm(z_ps, mqbh_row, onesr512, start=False, stop=True,
           skip_group_check=True)
        z_sb = pb.tile([H, SEG], bf16, tag="z_sb")
        nc.vector.tensor_scalar(z_sb, z_ps, 0.5, None, OP.mult)
        ztm_ps = psB.tile([CH, 4, H], bf16, tag="sc", bufs=1, name="ztm")
        for c in range(4):
            nc.tensor.transpose(ztm_ps[:, c, :],
                                z_sb[:, c * CH:(c + 1) * CH],
                                ident[0:H, 0:H])
        tht = pb.tile([CH, 4, H], f32, tag="tht")
        nc.scalar.activation(tht, ztm_ps, AF.Tanh)
        num = pb.tile([CH, 4, H], f32, tag="num")
        nc.vector.tensor_scalar_add(num, tht, 1.0)
        den = pb.tile([CH, 4, H], f32, tag="den")
        nc.vector.tensor_scalar(den, tht, -1.0, 1.0, OP.mult, OP.add)
        rec = pb.tile([CH, 4, H], f32, tag="rec")
        nc.vector.reciprocal(rec, den)
        ex = pb.tile([CH, 4, H], f32, tag="ex")
        nc.vector.tensor_tensor(ex, num, rec, OP.mult)
        es = pb.tile([CH, 4], f32, tag="es")
        nc.vector.tensor_reduce(es, ex, AX, OP.add)
        esr = pb.tile([CH, 4], f32, tag="esr")
        nc.vector.reciprocal(esr, es)
        at_tm = pb.tile([CH, 4, H], bf16, tag="at_tm")
        nc.vector.tensor_tensor(at_tm, ex,
                                esr.unsqueeze(-1).broadcast_to([CH, 4, H]),
                                OP.mult)
        atf_ps = psB.tile([H, SEG], bf16, tag="sc", bufs=1, name="atf")
        for c in range(4):
            nc.tensor.transpose(atf_ps[:, c * CH:(c + 1) * CH],
                                at_tm[:, c, :], ident)
        at = pb.tile([H, SEG], bf16, tag="at")
        nc.vector.tensor_copy(at, atf_ps)
        if os.environ.get("DEBUG_DUMP") and it == 0:
            da = consts.tile([H, SEG], f32, tag="dbg_a")
            nc.vector.tensor_copy(da, at)
            dma(out=t["d_at"], in_=da)
        sqa = pb.tile([H, SEG], bf16, tag="sqa")
        nc.gpsimd.tensor_tensor(sqa, at, at, OP.mult)

        # two half-segment pipelines: retrieval -> LN -> MLP -> gate,
        # h1's PE work overlaps h0's DVE/ACT chains
        rf = pb.tile([CH, 2, SEG], bf16, tag="rf")
        gd = pb.tile([CH, 2, SEG], bf16, tag="gd")
        for h in range(2):
            hsl = slice(h * HSEG, (h + 1) * HSEG)
            r_ps = psB.tile([CH, 2, HSEG], f32, tag="r", bufs=1,
                            name=f"r_ps{h}")
            for dd in range(2):
                # single accumulation group per PSUM bank: has_written is
                # per (partition, bank); only the first matmul may start
                mm(r_ps[:, dd, :], S_h[h][:, dd * CH:(dd + 1) * CH],
                   qfs_l[h], start=(dd == 0), stop=False,
                   skip_group_check=True)
            for bi, (kc, lo, n) in enumerate(HALF_BLOCKS[h]):
                qsl = slice(lo, lo + n)
                sc_ps = psB.tile([CH, 2 * CH], f32, tag="sc", bufs=1,
                                 name="sc_ps")
                mm(sc_ps[:, 0:n], kff[:, kc * CH:(kc + 1) * CH],
                   qfs_l[h][:, qsl], start=True, stop=True)
                sc_sb = pb.tile([CH, 2 * CH], bf16, tag="sc_sb")
                nc.vector.tensor_tensor(sc_sb[:, 0:CH], sc_ps[:, 0:CH],
                                        tril, OP.mult)
                if n > CH:
                    nc.vector.tensor_copy(sc_sb[:, CH:n], sc_ps[:, CH:n])
                last = bi == len(HALF_BLOCKS[h]) - 1
                for dd in range(2):
                    mm(r_ps[:, dd, qsl], v_own[:, kc, dd * CH:(dd + 1) * CH],
                       sc_sb[:, 0:n], start=False,
                       stop=(last and dd == 1), skip_group_check=True)

            rt = pb.tile([CH, 2, HSEG], bf16, tag="rt")
            nc.vector.tensor_copy(rt, r_ps)
            if os.environ.get("DEBUG_DUMP") and it == 0:
                drt = consts.tile([CH, 2, HSEG], f32, tag=f"dbg_rt{h}")
                nc.vector.tensor_copy(drt, rt)
                dma(out=t["d_rt"][:, :, h * HSEG:(h + 1) * HSEG], in_=drt)
            sq = pb.tile([CH, 2, HSEG], bf16, tag="sq")
            nc.gpsimd.tensor_tensor(sq, rt, rt, OP.mult)

            st_ps = psB.tile([33, HSEG], f32, tag="st", bufs=2,
                             name=f"st_ps{h}")
            mm(st_ps[0:1, :], oc264, rt[:, 0, :], start=True, stop=False,
               skip_group_check=True)
            mm(st_ps[0:1, :], oc264, rt[:, 1, :], start=False, stop=True,
               skip_group_check=True)
            mm(st_ps[32:33, :], oc264, sq[:, 0, :], start=True, stop=False,
               skip_group_check=True)
            mm(st_ps[32:33, :], oc264, sq[:, 1, :], start=False, stop=False,
               skip_group_check=True)
            mm(st_ps[32:33, :], oc264[0:H, :], sqa[:, hsl], start=False,
               stop=False, skip_group_check=True)
            mm(st_ps[32:33, :], onesr[0:1, 0:1], eps_row[:, 0:HSEG],
               start=False, stop=True, skip_group_check=True)
            m_b = pb.tile([1, HSEG], bf16, tag="m_b")
            nc.vector.tensor_scalar_add(m_b, st_ps[0:1, :], 1.0 / (D + H))
            msq = pb.tile([1, HSEG], f32, tag="lmsq")
            nc.vector.tensor_tensor(msq, m_b, m_b, OP.mult)
            rstd = ln_rstd(st_ps[32:33, :], msq, "l", HSEG)
            rbb = psB.tile([CH, HSEG], f32, tag="st", bufs=2,
                           name=f"rbb{h}")
            mm(rbb, onesr, rstd, start=True, stop=True)
            rb_sb = pb.tile([CH, HSEG], bf16, tag="rb_sb")
            nc.scalar.copy(rb_sb, rbb)

            cn = pb.tile([CH, 2, HSEG], bf16, tag="cn")
            for dd in range(2):
                nc.vector.tensor_tensor(cn[:, dd, :], rt[:, dd, :], rb_sb,
                                        OP.mult)
            if os.environ.get("DEBUG_DUMP") and it == 0:
                dcn = consts.tile([CH, 2, HSEG], f32, tag=f"dbg_cn{h}")
                nc.vector.tensor_copy(dcn, cn)
                dma(out=t["d_cn"][:, :, h * HSEG:(h + 1) * HSEG], in_=dcn)
                drs = consts.tile([1, HSEG], f32, tag=f"dbg_rs{h}")
                nc.vector.tensor_copy(drs, rstd)
                dma(out=t["d_rstd"][h:h + 1, 0:HSEG], in_=drs)
            cnam = pb.tile([33, HSEG], bf16, tag="cna")
            nc.vector.memset(cnam, 0.0)
            nc.vector.tensor_tensor(cnam[0:H, :], at[:, hsl], rb_sb[0:H, :],
                                    OP.mult)
            nc.vector.tensor_tensor(cnam[32:33, :], m_b, rstd, OP.mult)

            hh = pb.tile([CH, 4, HSEG], bf16, tag="h")
            for o in range(4):
                h_ps = psB.tile([CH, HSEG], f32, tag="hps", bufs=2,
                                name="h_ps")
                mm(h_ps, w1k(it, 0, o), cn[:, 0, :], start=True, stop=False)
                mm(h_ps, w1k29(it, o), cnam, start=False, stop=False)
                mm(h_ps, w1k(it, 1, o), cn[:, 1, :], start=False, stop=True)
                nc.scalar.activation(hh[:, o, :], h_ps, AF.Silu, scale=GSC,
                                     bias=cF[:, F_B1 + it * 4 + o:
                                             F_B1 + it * 4 + o + 1])
                if os.environ.get("DEBUG_DUMP") and it == 0:
                    dh = consts.tile([CH, HSEG], f32, tag=f"dbg_h{h}{o}")
                    nc.vector.tensor_copy(dh, hh[:, o, :])
                    dma(out=t["d_hh"][:, o, h * HSEG:(h + 1) * HSEG], in_=dh)

            for mh in range(2):
                rf_ps = psB.tile([CH, HSEG], f32, tag="hps", bufs=2,
                                 name="rf_ps")
                for kh in range(4):
                    mm(rf_ps, w2k(it, kh, mh), hh[:, kh, :],
                       start=(kh == 0), stop=(kh == 3))
                nc.scalar.activation(rf[:, mh, hsl], rf_ps, AF.Identity,
                                     bias=cF[:, F_B2 + it * 2 + mh:
                                             F_B2 + it * 2 + mh + 1])
            nc.vector.tensor_tensor(acc_l[h], acc_l[h],
                                    rf[:, :, hsl], OP.add)

            if it < I - 1:
                for mh in range(2):
                    g_ps = psB.tile([CH, HSEG], f32, tag="hps", bufs=2,
                                    name="g_ps")
                    for kh in range(4):
                        rhs = (q[:, kh, hsl] if kh < 2
                               else rf[:, kh - 2, hsl])
                        mm(g_ps, gwk(it, kh, mh), rhs,
                           start=(kh == 0), stop=(kh == 3))
                    nc.scalar.activation(gd[:, mh, hsl], g_ps, AF.Tanh,
                                         bias=cF[:, F_GB + it * 2 + mh:
                                                 F_GB + it * 2 + mh + 1])
        if it < I - 1:
            nc.vector.tensor_tensor(qn, q, gd, OP.add)
        if os.environ.get("DEBUG_DUMP") and it == 0:
            drf = consts.tile([CH, 2, SEG], f32, tag="dbg_rf")
            nc.vector.tensor_copy(drf, rf)
            dma(out=t["d_rf"], in_=drf)
            dq1 = consts.tile([CH, 2, SEG], f32, tag="dbg_q1")
            nc.vector.tensor_copy(dq1, qn)
            dma(out=t["d_q1"], in_=dq1)

    # =========== final LN(acc) @ wog + x (+boe), per token-half ===========
    yt = pb.tile([CH, 2, SEG], f32, tag="yt")
    for h in range(2):
        hsl = slice(h * HSEG, (h + 1) * HSEG)
        sqf = pb.tile([CH, 2, HSEG], bf16, tag="sq")
        nc.scalar.activation(sqf, acc_l[h], AF.Square)
        stf = psB.tile([33, HSEG], f32, tag="st", bufs=2, name=f"stf{h}")
        mm(stf[0:1, :], oc256, acc_l[h][:, 0, :], start=True, stop=False,
           skip_group_check=True)
        mm(stf[0:1, :], oc256, acc_l[h][:, 1, :], start=False, stop=True,
           skip_group_check=True)
        mm(stf[32:33, :], oc256, sqf[:, 0, :], start=True, stop=False,
           skip_group_check=True)
        mm(stf[32:33, :], oc256, sqf[:, 1, :], start=False, stop=False,
           skip_group_check=True)
        mm(stf[32:33, :], onesr[0:1, 0:1], eps_row[:, 0:HSEG],
           start=False, stop=True, skip_group_check=True)
        m_f = pb.tile([1, HSEG], bf16, tag="m_b")
        nc.vector.tensor_copy(m_f, stf[0:1, :])
        msqf = pb.tile([1, HSEG], f32, tag="lmsq")
        nc.vector.tensor_tensor(msqf, m_f, m_f, OP.mult)
        rstdf = ln_rstd(stf[32:33, :], msqf, "l", HSEG)
        mrf = pb.tile([1, HSEG], bf16, tag="mr")
        nc.vector.tensor_tensor(mrf, m_f, rstdf, OP.mult)
        rbf = psB.tile([CH, HSEG], f32, tag="st", bufs=2, name=f"rbf{h}")
        mm(rbf, onesr, rstdf, start=True, stop=True)
        rbf_sb = pb.tile([CH, HSEG], bf16, tag="rb_sb")
        nc.scalar.copy(rbf_sb, rbf)
        cnf = pb.tile([CH, 2, HSEG], bf16, tag="cn")
        for dd in range(2):
            nc.vector.tensor_tensor(cnf[:, dd, :], acc_l[h][:, dd, :],
                                    rbf_sb, OP.mult)
        for mh in range(2):
            o_ps = psB.tile([CH, HSEG], f32, tag="hps", bufs=2, name="o_ps")
            mm(o_ps, wog(0, mh), cnf[:, 0, :], start=True, stop=False)
            mm(o_ps, wog(1, mh), cnf[:, 1, :], start=False, stop=False)
            mm(o_ps, wogsum(mh), mrf, start=False, stop=True)
            nc.vector.tensor_tensor(yt[:, mh, hsl], o_ps, xfm_sb[:, mh, hsl],
                                    OP.add)
    if not os.environ.get("DEBUG_RT"):
        dma(out=t["y"], in_=yt)

    for pool in (psB, pb, pa, own, consts):
        pool.release()
    lp.__exit__(None, None, None)


def _prep_inputs(inputs):
    """Host-side parameter folding + per-core input maps."""
    f = lambda a: np.asarray(a, dtype=np.float32)
    tobf = lambda a: np.ascontiguousarray(
        np.asarray(a, dtype=np.float32)).astype(ml_dtypes.bfloat16)
    x = f(inputs["x"])
    pe_w, pe_b = f(inputs["pe_w"]), f(inputs["pe_b"])
    tv_w, tv_b = f(inputs["tv_w"]), f(inputs["tv_b"])
    mq_w, mq_b = f(inputs["mq_w"]), f(inputs["mq_b"])
    ln_g, ln_b = f(inputs["ref_ln_g"]), f(inputs["ref_ln_b"])
    w1, b1 = f(inputs["ref_w1"]), f(inputs["ref_b1"])
    w2, b2 = f(inputs["ref_w2"]), f(inputs["ref_b2"])
    gw, gb = f(inputs["gate_w"]), f(inputs["gate_b"])
    og, ob = f(inputs["out_ln_g"]), f(inputs["out_ln_b"])
    ow, obias = f(inputs["out_w"]), f(inputs["out_b"])

    w1g = ln_g[:, :, None] * w1                       # (I, 264, 512)
    b1e = b1 + np.einsum("if,ifo->io", ln_b, w1)      # (I, 512)
    w2s = w2 / GSC
    wogm = og[:, None] * ow                           # (256, 256)
    boe = obias + ob @ ow

    # ---- blobA ----
    blobA = np.zeros((CH, A_COLS), np.float32)
    tvpe = np.concatenate([tv_w, pe_w], axis=1)       # (256, 288)
    blobA[:, A_TVPE:A_TVPE + 288] = tvpe[0:128]
    blobA[:, A_TVPE + 288:A_TVPE + 576] = tvpe[128:256]
    blobA[:, A_ID:A_ID + CH] = np.eye(CH)
    blobA[:, A_TRIL:A_TRIL + CH] = np.triu(np.ones((CH, CH)))
    blobA[:, A_OC264] = 1.0 / (D + H)
    blobA[:, A_OC256] = 1.0 / D
    blobA[:, A_ONEC] = 1.0
    blobA[0, A_ONER:A_ONER + 4 * CH] = 1.0
    blobA[0, A_PEBR:A_PEBR + P] = pe_b
    blobA[0, A_MQBH:A_MQBH + H] = mq_b

    # ---- blobB ----
    blobB = np.zeros((CH, B_COLS), np.float32)
    for kh in range(2):
        blobB[:, B_PEW + kh * P:B_PEW + (kh + 1) * P] = \
            pe_w[kh * CH:(kh + 1) * CH]
        blobB[:, B_MQW + kh * H:B_MQW + (kh + 1) * H] = \
            mq_w[kh * CH:(kh + 1) * CH]
        for mh in range(2):
            blobB[:, B_WOG + (kh * 2 + mh) * CH:
                  B_WOG + (kh * 2 + mh + 1) * CH] = \
                wogm[kh * CH:(kh + 1) * CH, mh * CH:(mh + 1) * CH]
    for it in range(I):
        for kh in range(2):
            blobB[:, B_W1K + it * 1024 + kh * 512:
                  B_W1K + it * 1024 + (kh + 1) * 512] = \
                w1g[it, kh * CH:(kh + 1) * CH, :]
        for kh in range(4):
            for mh in range(2):
                blobB[:, B_W2K + it * 1024 + (kh * 2 + mh) * CH:
                      B_W2K + it * 1024 + (kh * 2 + mh + 1) * CH] = \
                    w2s[it, kh * CH:(kh + 1) * CH, mh * CH:(mh + 1) * CH]
        blobB[0:H, B_W1K2 + it * 512:B_W1K2 + (it + 1) * 512] = \
            w1g[it, D:D + H, :]
        blobB[32, B_W1K2 + it * 512:B_W1K2 + (it + 1) * 512] = \
            -w1g[it].sum(axis=0)
    for it in range(2):
        for kh in range(4):
            for mh in range(2):
                blobB[:, B_GWK + it * 1024 + (kh * 2 + mh) * CH:
                      B_GWK + it * 1024 + (kh * 2 + mh + 1) * CH] = \
                    gw[it, kh * CH:(kh + 1) * CH, mh * CH:(mh + 1) * CH]
    blobB[0, B_WOGSUM:B_WOGSUM + D] = -wogm.sum(axis=0)

    # ---- blobF (fp32) ----
    blobF = np.zeros((CH, F_COLS), np.float32)
    blobF[0:P, F_PEB] = pe_b
    blobF[0:H, F_MQB] = 0.5 * mq_b
    for it in range(I):
        blobF[:, F_B1 + it * 4:F_B1 + (it + 1) * 4] = \
            (GSC * b1e[it]).reshape(4, CH).T
        blobF[:, F_B2 + it * 2:F_B2 + (it + 1) * 2] = \
            b2[it].reshape(2, CH).T
    for it in range(2):
        blobF[:, F_GB + it * 2:F_GB + (it + 1) * 2] = \
            gb[it].reshape(2, CH).T
    blobF[:, F_TRIL:F_TRIL + CH] = np.triu(np.ones((CH, CH)))

    blobA = tobf(blobA)
    blobB = tobf(blobB)
    tvb = np.ascontiguousarray(tv_b[None, :])

    in_maps = []
    for core in range(NCORES):
        b, pos = divmod(core, NCORES // B)
        s0 = pos * SEG
        xb = x[b]                                     # (L, D)
        xp = np.zeros((L, D), np.float32)
        xp[0:s0] = xb[0:s0]
        xp[NPREF * CH:NSLOT * CH] = xb[s0:s0 + SEG]
        xp_fm = xp.T                                  # (D, 2048)
        blobC = np.zeros((CH, C_COLS), np.float32)
        blobC[:, C_X:C_X + 2048] = xp_fm[0:CH]
        blobC[:, C_X + 2048:C_X + 4096] = xp_fm[CH:2 * CH]
        gl = np.arange(s0, s0 + SEG, dtype=np.float64)
        iv = (1.0 / (np.sqrt(gl + 1.0) * math.sqrt(P))).astype(np.float32)
        blobC[0:2 * P, C_INVN:C_INVN + SEG] = iv[None, :]
        km = np.zeros(NSLOT, np.float32)
        km[0:4 * pos] = 1.0
        km[NPREF:] = 1.0
        blobC[:, C_KM:C_KM + NSLOT] = km[None, :]
        x_fm = np.zeros((CH, 2, SEG), np.float32)
        xo = xb[s0:s0 + SEG] + boe[None, :]           # (512, 256)
        x_fm[:, 0, :] = xo.T[0:CH]
        x_fm[:, 1, :] = xo.T[CH:2 * CH]
        m = {"blobA_" + _SALT: blobA, "blobB": blobB, "blobF": blobF,
             "tvb": tvb, "x_fm": np.ascontiguousarray(x_fm),
             "blobC": tobf(blobC)}
        in_maps.append(m)
    return in_maps


def kernel(**inputs):
    from concourse.bass_utils import run_bass_kernel_spmd

    if "nc" not in _CACHE:
        _CACHE["nc"] = _build_program()
    nc = _CACHE["nc"]
    in_maps = _prep_inputs(inputs)
    res = run_bass_kernel_spmd(nc, in_maps, core_ids=list(range(NCORES)))
    out = np.empty((B, L, D), dtype=np.float32)
    for core in range(NCORES):
        b, pos = divmod(core, NCORES // B)
        s0 = pos * SEG
        y = np.asarray(res.results[core]["y"])        # (128, 2, 512)
        out[b, s0:s0 + SEG, :] = y.transpose(1, 0, 2).reshape(D, SEG).T
    return out


def gather(res):
    out = np.empty((B, L, D), dtype=np.float32)
    for core in range(NCORES):
        b, pos = divmod(core, NCORES // B)
        s0 = pos * SEG
        y = np.asarray(res.results[core]["y"])
        out[b, s0:s0 + SEG, :] = y.transpose(1, 0, 2).reshape(D, SEG).T
    return out


# revision 37
# speedup vs baseline: 1.0887x; 1.0887x over previous
"""Trainium2 Bass kernel for nn_AttentionGuidedIterativeBlock.

Math reformulation: the (B,L,P,D) phasor cumsum + retrieval is causal linear
attention with feature map Kf = [cos(phases), sin(phases)] (2P=64 dims):

    retrieved[l] = (sum_{l'<=l} (Qf[l].Kf[l']) * V[l']) / (sqrt(l+1)*sqrt(P))

Sharding: 8 cores x 512 tokens (cores 0-3 batch 0, 4-7 batch 1).  Each core
rebuilds the prefix state S = Kf^T @ V_aug over its batch prefix from a
per-core slot-permuted copy of x (12 prefix slots, zero-padded + masked via a
per-slot km scalar, then the 4 own chunks), so the program is uniform across
cores while own-segment K/V prep and prefix-state accumulation share the same
matmuls.  Everything runs in bf16 on the PE (fp32 PSUM accumulate); LayerNorm
mean-subtraction is folded into w1/wog as a rank-1 matmul (-mr x colsum(W)),
gelu(exact erf) is replaced by the sigmoid approximation expressed as
silu(1.702x)/1.702 so every activation function used stays inside one
activation-table set except nothing -- zero table swaps except none.
"""

import math
import os

import numpy as np
import ml_dtypes

D, P, I, H = 256, 32, 3, 8
B, L = 2, 2048
NCORES = 8
SEG = 512          # tokens per core
CH = 128           # chunk (tile partition) size
NSLOT = 16         # 12 prefix slots + 4 own chunks
NPREF = 12
HSEG = 256
PI = math.pi
EPS = 1e-5
GSC = 1.702        # sigmoid-gelu: gelu(x) ~= silu(GSC*x)/GSC

_CACHE = {}

# Source-content salt folded into a tensor name: any kernel.py change makes a
# distinct NEFF signature, so the neuron compile cache can never serve a NEFF
# built from a different version of this file.
import hashlib
with open(__file__, "rb") as _f:
    _SALT = hashlib.md5(_f.read()).hexdigest()[:8]

# ---- shared-blob column offsets (bf16, 128 partitions) ----
# blobA (early consts): tvpe (2,288), ident(128), tril(128), oc264, oc256,
#   onescol, onesrow(row0,128), pe_b_row(row0,32)
A_TVPE = 0
A_ID = 576
A_TRIL = 704
A_OC264 = 832
A_OC256 = 833
A_ONEC = 834
A_ONER = 835          # row0 cols 835:1347 = 1.0
A_PEBR = 1347         # row0 cols 1347:1379
A_MQBH = 1379         # row0 cols 1379:1387 = 0.5*mq_b
A_COLS = 1387

# blobB (weights): pe_w (2,32), mq_w (2,8), wog (2,2,128), w1k I*(2,512),
#   w2k I*(4,2,128), gwk 2*(4,2,128), w1k2 I*(8rows,512), w1sum_neg I*(row0,512),
#   wogsum_neg (row0, 256)
B_PEW = 0
B_MQW = 64
B_WOG = 80
B_W1K = 592
B_W2K = B_W1K + I * 1024          # 3664
B_GWK = B_W2K + I * 1024          # 6736
B_W1K2 = B_GWK + 2 * 1024         # 8784 (rows 0:8)
B_W1SUM = B_W1K2 + I * 512        # 10320 (row0)
B_WOGSUM = B_W1SUM + I * 512      # 11856 (row0)
B_COLS = B_WOGSUM + 256           # 12112

# blobF (fp32 biases): pe_b_col(32,1), mq_bh_col(8,1), b1s I*(128,4),
#   b2 I*(128,2), gb 2*(128,2)
F_PEB = 0
F_MQB = 1
F_B1 = 2
F_B2 = F_B1 + I * 4       # 14
F_GB = F_B2 + I * 2       # 20
F_TRIL = F_GB + 4         # 24
F_COLS = F_TRIL + 128     # 152

# blobC (per-core): x_perm (2,2048), invn (rows0:64, 512), km (16)
C_X = 0
C_INVN = 4096
C_KM = 4608
C_COLS = 4624


def _patch_walrus_passes():
    import concourse.bass_utils as bu
    if getattr(bu, "_nv_patched", False):
        return
    orig = bu.run_command

    def patched(cmd, cwd=None, **kw):
        cmd = list(cmd)
        if "--pass" in cmd:
            i = cmd.index("--pass")
            cmd[i + 1] = cmd[i + 1].replace("birverifier,", "")
        return orig(cmd, cwd=cwd, **kw)

    bu.run_command = patched
    bu._nv_patched = True


def _build_program(split=True):
    _patch_walrus_passes()
    import concourse.bass as bass
    import concourse.tile as tile
    from concourse import mybir

    f32 = mybir.dt.float32

    nc = bass.Bass("TRN2", target_bir_lowering=False, debug=False,
                   num_devices=NCORES)

    def din(name, shape, dt):
        return nc.dram_tensor(name, shape, dt, kind="ExternalInput").ap()

    bf16 = mybir.dt.bfloat16
    t = {}
    t["blobA_" + _SALT] = din("blobA_" + _SALT, (CH, A_COLS), bf16)
    t["blobA"] = t["blobA_" + _SALT]
    t["blobB"] = din("blobB", (CH, B_COLS), bf16)
    t["blobC"] = din("blobC", (CH, C_COLS), bf16)
    t["blobF"] = din("blobF", (CH, F_COLS), f32)
    t["tvb"] = din("tvb", (1, D), f32)
    t["x_fm"] = din("x_fm", (CH, 2, SEG), f32)
    t["y"] = nc.dram_tensor("y", (CH, 2, SEG), f32, kind="ExternalOutput").ap()
    if os.environ.get("DEBUG_DUMP"):
        def dout(name, shape):
            t[name] = nc.dram_tensor(name, shape, f32,
                                     kind="ExternalOutput").ap()
        dout("d_qfs", (2 * P, SEG))
        dout("d_at", (H, SEG))
        dout("d_rt", (CH, 2, SEG))
        dout("d_cn", (CH, 2, SEG))
        dout("d_rstd", (2, SEG))
        dout("d_hh", (CH, 4, SEG))
        dout("d_rf", (CH, 2, SEG))
        dout("d_q1", (CH, 2, SEG))
        dout("d_sh", (2 * P, 2, D))

    with tile.TileContext(nc) as tc:
        _body(tc, nc, t, f32, bf16, bass, mybir)
    if split:
        _split_waits(nc, mybir)
    return nc


def _split_waits(nc, mybir, cap=1):
    """Move excess sync-waits onto preceding same-engine NOPs (this walrus
    build allows one wait slot per instruction)."""
    for fn in nc.m.functions:
        for blk in fn.blocks:
            out = []
            for ins in blk.instructions:
                si = ins.sync_info
                if si is not None and len(si.on_wait) > cap:
                    waits = list(si.on_wait)
                    extra, keep = waits[:-cap], waits[-cap:]
                    for j, w in enumerate(extra):
                        nop = mybir.InstNoOp(name=f"{ins.name}_wsplit{j}",
                                             ins=[], outs=[])
                        nop.engine = ins.engine
                        nop.sync_info = mybir.SyncInfo(on_wait=[w],
                                                       on_update=[])
                        out.append(nop)
                    ins.sync_info = mybir.SyncInfo(on_wait=keep,
                                                   on_update=si.on_update)
                out.append(ins)
            blk.instructions = out


def _body(tc, nc, t, f32, bf16, bass, mybir):
    from concourse.alu_op_type import AluOpType as OP

    AF = mybir.ActivationFunctionType
    AX = mybir.AxisListType.X

    lp = nc.allow_low_precision(reason="bf16 kernel; tolerance 2e-2 validated")
    lp.__enter__()
    consts = tc.alloc_tile_pool(name="consts", bufs=1)
    own = tc.alloc_tile_pool(name="own", bufs=1)
    pa = tc.alloc_tile_pool(name="pa", bufs=2)
    pb = tc.alloc_tile_pool(name="pb", bufs=2)

    dma = nc.sync.dma_start
    mm = nc.tensor.matmul

    # ---- constant blobs (6 input DMAs total) ----
    cC = consts.tile([CH, C_COLS], bf16)
    dma(out=cC, in_=t["blobC"])
    cA = consts.tile([CH, A_COLS], bf16)
    dma(out=cA, in_=t["blobA"])
    cB = consts.tile([CH, B_COLS], bf16)
    dma(out=cB, in_=t["blobB"])
    cF = consts.tile([CH, F_COLS], f32)
    dma(out=cF, in_=t["blobF"])
    tvb_bc = consts.tile([2 * P, D], f32)
    dma(out=tvb_bc, in_=t["tvb"].to_broadcast((2 * P, D)))
    xfm_sb = consts.tile([CH, 2, SEG], f32)
    dma(out=xfm_sb, in_=t["x_fm"])

    def xslot(kh, s):
        return cC[:, C_X + kh * 2048 + s * CH: C_X + kh * 2048 + (s + 1) * CH]

    tvpe = lambda kh: cA[:, A_TVPE + kh * 288: A_TVPE + (kh + 1) * 288]
    ident = cA[:, A_ID:A_ID + CH]
    tril = cF[:, F_TRIL:F_TRIL + CH]
    oc264 = cA[:, A_OC264:A_OC264 + 1]
    oc256 = cA[:, A_OC256:A_OC256 + 1]
    onesr = cA[0:1, A_ONER:A_ONER + CH]
    onesr512 = cA[0:1, A_ONER:A_ONER + SEG]
    mqbh_row = cA[0:1, A_MQBH:A_MQBH + H]
    pebr = cA[0:1, A_PEBR:A_PEBR + P]
    invn = cC[0:2 * P, C_INVN:C_INVN + SEG]
    pe_b_col = cF[0:P, F_PEB:F_PEB + 1]
    mq_bh_col = cF[0:H, F_MQB:F_MQB + 1]
    halfpi = consts.tile([CH, 1], f32)
    nc.vector.memset(halfpi, PI / 2)
    i32 = mybir.dt.int32
    i16 = mybir.dt.int16
    qmagic16 = consts.tile([1, SEG], i16)
    nc.vector.memset(qmagic16, 0x5F37)
    ones8f = consts.tile([H, 1], f32)
    nc.vector.memset(ones8f, 1.0)
    ones1x8f = consts.tile([1, H], f32)
    nc.vector.memset(ones1x8f, 1.0)
    eps_row = consts.tile([1, SEG], bf16)
    nc.vector.memset(eps_row, EPS)
    c264 = consts.tile([1, 1], f32)
    nc.vector.memset(c264, 1.0 / (D + H))
    f32r = mybir.dt.float32r
    rr = lambda ap: ap.bitcast(f32r)

    def ln_rstd(st2_psum, msq, tagp, ln=SEG):
        # rstd = 1/sqrt(st2 - m^2), DVE-only (bf16 Quake seed + 1 Newton,
        # all ops at the 2x 16-bit DVE rate); eps pre-accumulated into st2.
        var = pb.tile([1, ln], bf16, tag=tagp + "var")
        nc.vector.tensor_tensor(var, st2_psum, msq, OP.subtract)
        sh = pb.tile([1, ln], i16, tag=tagp + "sh")
        nc.vector.tensor_scalar(sh, var.bitcast(i16), 1, None,
                                OP.logical_shift_right)
        si = pb.tile([1, ln], i16, tag=tagp + "si")
        nc.vector.tensor_tensor(si, qmagic16[:, 0:ln], sh, OP.subtract)
        y = si.bitcast(bf16)
        y2 = pb.tile([1, ln], bf16, tag=tagp + "y2")
        nc.vector.tensor_tensor(y2, y, y, OP.mult)
        tn = pb.tile([1, ln], bf16, tag=tagp + "tn")
        nc.vector.tensor_tensor(tn, y2, var, OP.mult)
        un = pb.tile([1, ln], bf16, tag=tagp + "un")
        nc.vector.tensor_scalar(un, tn, -0.5, 1.5, OP.mult, OP.add)
        rstd = pb.tile([1, ln], bf16, tag=tagp + "rstd")
        nc.vector.tensor_tensor(rstd, y, un, OP.mult)
        return rstd

    pe_w = lambda kh: cB[:, B_PEW + kh * P: B_PEW + (kh + 1) * P]
    mq_w = lambda kh: cB[:, B_MQW + kh * H: B_MQW + (kh + 1) * H]
    wog = lambda kh, mh: cB[:, B_WOG + (kh * 2 + mh) * CH:
                            B_WOG + (kh * 2 + mh + 1) * CH]
    w1k = lambda it, kh, o: cB[:, B_W1K + it * 1024 + kh * 512 + o * CH:
                               B_W1K + it * 1024 + kh * 512 + (o + 1) * CH]
    w2k = lambda it, kh, mh: cB[:, B_W2K + it * 1024 + (kh * 2 + mh) * CH:
                                B_W2K + it * 1024 + (kh * 2 + mh + 1) * CH]
    gwk = lambda it, kh, mh: cB[:, B_GWK + it * 1024 + (kh * 2 + mh) * CH:
                                B_GWK + it * 1024 + (kh * 2 + mh + 1) * CH]
    w1k29 = lambda it, o: cB[0:33, B_W1K2 + it * 512 + o * CH:
                             B_W1K2 + it * 512 + (o + 1) * CH]
    wogsum = lambda mh: cB[0:1, B_WOGSUM + mh * CH:B_WOGSUM + (mh + 1) * CH]

    # =========== phase A: prefix state + own K/V (unified slot loop) =======
    psA = tc.alloc_tile_pool(name="psA", bufs=1, space="PSUM")


    S_ps = psA.tile([2 * P, D + 8], f32, tag="S")
    own_kf = own.tile([CH, 4, 2 * P], bf16)    # own kf, token-major, unmasked
    v_own = own.tile([CH, 4, D + 8], bf16)     # own V_aug, token-major

    for g in range(8):
        # 2-slot groups with double-buffered PSUM keep the PE gap-free
        vq_ps = psA.tile([CH, 2, 512], f32, tag="vq", bufs=2,
                         name=f"vq{g}")
        for c in range(2):
            s = 2 * g + c
            mm(vq_ps[:, c, 0:D + P], xslot(0, s), tvpe(0),
               start=True, stop=False, skip_group_check=True)
            mm(vq_ps[:, c, 0:D + P], xslot(1, s), tvpe(1),
               start=False, stop=False, skip_group_check=True)
            mm(vq_ps[:, c, D:D + P], onesr, pebr, start=False, stop=True,
               skip_group_check=True)
        tq = pa.tile([CH, 2, P], bf16, tag="tq")
        nc.scalar.activation(tq, vq_ps[:, :, D:D + P], AF.Tanh)
        aq = pa.tile([CH, 2, P], bf16, tag="aq")
        nc.scalar.activation(aq, tq, AF.Abs)
        j = 2 * (g - 6)
        kfg = (own_kf[:, j:j + 2, :] if g >= 6
               else pa.tile([CH, 2, 2 * P], bf16, tag="kf"))
        nc.scalar.activation(kfg[:, :, 0:P], aq, AF.Sin, scale=-PI,
                             bias=halfpi)
        nc.scalar.activation(kfg[:, :, P:2 * P], tq, AF.Sin, scale=PI)
        vdst = (v_own[:, j:j + 2, :] if g >= 6
                else pa.tile([CH, 2, D + 8], bf16, tag="vsb"))
        nc.vector.tensor_copy(vdst[:, :, 0:D], vq_ps[:, :, 0:D])
        nc.vector.memset(vdst[:, :, D:D + 8], 1.0)
        if g < 6:
            kfm = pa.tile([CH, 2, 2 * P], bf16, tag="kfm")
            nc.vector.tensor_tensor(
                kfm, kfg,
                cC[:, C_KM + 2 * g:C_KM + 2 * g + 2].unsqueeze(-1)
                .broadcast_to([CH, 2, 2 * P]), OP.mult)
            for c in range(2):
                s = 2 * g + c
                mm(S_ps, kfm[:, c, :], vdst[:, c, :],
                   start=(s == 0), stop=(s == NPREF - 1),
                   skip_group_check=True)

    # S_h0 = prefix state (+ rank-1 tv_b fold);  S_h1 adds own chunks 0,1
    S_h = []
    for hi in range(2):
        if hi == 1:
            for c in range(2):
                mm(S_ps, own_kf[:, c, :], v_own[:, c, :],
                   start=False, stop=(c == 1), skip_group_check=True)
        kfsum = own.tile([2 * P, 1], f32, tag=f"kfsum{hi}")
        nc.vector.tensor_copy(kfsum, S_ps[:, D:D + 1])
        st = own.tile([2 * P, D], f32, tag=f"S_tmp{hi}")
        nc.vector.tensor_tensor(
            st, tvb_bc, kfsum.broadcast_to([2 * P, D]), OP.mult)
        sh = own.tile([2 * P, D], bf16, tag=f"S_h{hi}")
        nc.vector.tensor_tensor(sh, st, S_ps[:, 0:D], OP.add)
        S_h.append(sh)

    # kff: own kf feature-major (for intra scores + iteration-0 Qf)
    kff = own.tile([2 * P, SEG], bf16)
    for c in range(4):
        tr_ps = psA.tile([2 * P, CH], bf16, tag="tr", bufs=2, name="tr_ps")
        nc.tensor.transpose(tr_ps, own_kf[:, c, :], ident)
        nc.vector.tensor_copy(kff[:, c * CH:(c + 1) * CH], tr_ps)

    acc_l = []
    for h in range(2):
        acc_h = own.tile([CH, 2, HSEG], bf16, tag=f"acc{h}",
                         name=f"acc{h}")
        nc.vector.memset(acc_h, 0.0)
        acc_l.append(acc_h)
    qA = own.tile([CH, 2, SEG], bf16)
    qB = own.tile([CH, 2, SEG], bf16)
    # initial query = own x (bf16, feature-major) = slots 12-15 of x_perm
    for kh in range(2):
        nc.vector.tensor_copy(
            qA[:, kh, :],
            cC[:, C_X + kh * 2048 + NPREF * CH:C_X + kh * 2048 + NSLOT * CH])

    psA.release()
    psB = tc.alloc_tile_pool(name="psB", bufs=1, space="PSUM")

    # intra score blocks per half: (key chunk, local query lo, n)
    HALF_BLOCKS = {0: [(0, 0, 2 * CH), (1, CH, CH)],
                   1: [(2, 0, 2 * CH), (3, CH, CH)]}

    # =========== refinement iterations (full-width, feature-major) ========
    for it in range(I):
        q = qA if it % 2 == 0 else qB
        qn = qB if it % 2 == 0 else qA

        # Qf (feature-major), per token-half so h0's retrieval overlaps
        # h1's activation chain; 1/norm folded
        qfs_l = []
        for qh in range(2):
            qsl = slice(qh * HSEG, (qh + 1) * HSEG)
            if it > 0:
                if qh == 0:
                    qp_ps = psB.tile([P, SEG], f32, tag="mix", bufs=2,
                                     name="qp_ps")
                    mm(qp_ps, pe_w(0), q[:, 0, :], start=True, stop=False)
                    mm(qp_ps, pe_w(1), q[:, 1, :], start=False, stop=True)
                tq_ = pb.tile([P, HSEG], bf16, tag="tq")
                nc.scalar.activation(tq_, qp_ps[:, qsl], AF.Tanh,
                                     bias=pe_b_col)
                aq_ = pb.tile([P, HSEG], bf16, tag="aq")
                nc.scalar.activation(aq_, tq_, AF.Abs)
                qf_h = pb.tile([2 * P, HSEG], bf16, tag="qf")
                nc.scalar.activation(qf_h[0:P, :], aq_, AF.Sin, scale=-PI,
                                     bias=halfpi[0:P, :])
                nc.scalar.activation(qf_h[P:2 * P, :], tq_, AF.Sin, scale=PI)
            else:
                qf_h = kff[:, qsl]
            qfs_h = pb.tile([2 * P, HSEG], bf16, tag=f"qfs{qh}")
            nc.vector.tensor_tensor(qfs_h, qf_h, invn[:, qsl], OP.mult)
            qfs_l.append(qfs_h)
        if os.environ.get("DEBUG_DUMP") and it == 0:
            dq = consts.tile([2 * P, SEG], f32, tag="dbg_q")
            nc.vector.tensor_copy(dq, qfs)
            dma(out=t["d_qfs"], in_=dq)
            ds0 = consts.tile([2 * P, 2, D], f32, tag="dbg_s")
            nc.vector.tensor_copy(ds0[:, 0, :], S_h[0])
            nc.vector.tensor_copy(ds0[:, 1, :], S_h[1])
            dma(out=t["d_sh"], in_=ds0)

        # attention: feature-major logits, token-major tanh-softmax
        # (keeps reciprocals at 8/32 elems per DVE lane), mq_b/2 folded in
        # via a K=1 rank-1 matmul
        z_ps = psB.tile([H, SEG], f32, tag="mix", bufs=2, name="z_ps")
        mm(z_ps, mq_w(0), q[:, 0, :], start=True, stop=False,
           skip_group_check=True)
        mm(z_ps, mq_w(1), q[:, 1, :], start=False, stop=False,
           skip_group_check=True)
        mm(z_ps, mqbh_row, onesr512, start=False, stop=True,
           skip_group_check=True)
        z_sb = pb.tile([H, SEG], bf16, tag="z_sb")
        nc.vector.tensor_scalar(z_sb, z_ps, 0.5, None, OP.mult)
        ztm_ps = psB.tile([CH, 4, H], bf16, tag="sc", bufs=1, name="ztm")
        for c in range(4):
            nc.tensor.transpose(ztm_ps[:, c, :],
                                z_sb[:, c * CH:(c + 1) * CH],
                                ident[0:H, 0:H])
        tht = pb.tile([CH, 4, H], f32, tag="tht")
        nc.scalar.activation(tht, ztm_ps, AF.Tanh)
        num = pb.tile([CH, 4, H], f32, tag="num")
        nc.vector.tensor_scalar_add(num, tht, 1.0)
        den = pb.tile([CH, 4, H], f32, tag="den")
        nc.vector.tensor_scalar(den, tht, -1.0, 1.0, OP.mult, OP.add)
        rec = pb.tile([CH, 4, H], f32, tag="rec")
        nc.vector.reciprocal(rec, den)
        ex = pb.tile([CH, 4, H], f32, tag="ex")
        nc.vector.tensor_tensor(ex, num, rec, OP.mult)
        es = pb.tile([CH, 4], f32, tag="es")
        nc.vector.tensor_reduce(es, ex, AX, OP.add)
        esr = pb.tile([CH, 4], f32, tag="esr")
        nc.vector.reciprocal(esr, es)
        at_tm = pb.tile([CH, 4, H], bf16, tag="at_tm")
        nc.vector.tensor_tensor(at_tm, ex,
                                esr.unsqueeze(-1).broadcast_to([CH, 4, H]),
                                OP.mult)
        atf_ps = psB.tile([H, SEG], bf16, tag="sc", bufs=1, name="atf")
        for c in range(4):
            nc.tensor.transpose(atf_ps[:, c * CH:(c + 1) * CH],
                                at_tm[:, c, :], ident)
        at = pb.tile([H, SEG], bf16, tag="at")
        nc.vector.tensor_copy(at, atf_ps)
        if os.environ.get("DEBUG_DUMP") and it == 0:
            da = consts.tile([H, SEG], f32, tag="dbg_a")
            nc.vector.tensor_copy(da, at)
            dma(out=t["d_at"], in_=da)
        sqa = pb.tile([H, SEG], bf16, tag="sqa")
        nc.gpsimd.tensor_tensor(sqa, at, at, OP.mult)

        # two half-segment pipelines: retrieval -> LN -> MLP -> gate,
        # h1's PE work overlaps h0's DVE/ACT chains
        rf = pb.tile([CH, 2, SEG], bf16, tag="rf")
        gd = pb.tile([CH, 2, SEG], bf16, tag="gd")
        for h in range(2):
            hsl = slice(h * HSEG, (h + 1) * HSEG)
            r_ps = psB.tile([CH, 2, HSEG], f32, tag="r", bufs=1,
                            name=f"r_ps{h}")
            for dd in range(2):
                # single accumulation group per PSUM bank: has_written is
                # per (partition, bank); only the first matmul may start
                mm(r_ps[:, dd, :], S_h[h][:, dd * CH:(dd + 1) * CH],
                   qfs_l[h], start=(dd == 0), stop=False,
                   skip_group_check=True)
            for bi, (kc, lo, n) in enumerate(HALF_BLOCKS[h]):
                qsl = slice(lo, lo + n)
                sc_ps = psB.tile([CH, 2 * CH], f32, tag="sc", bufs=1,
                                 name="sc_ps")
                mm(sc_ps[:, 0:n], kff[:, kc * CH:(kc + 1) * CH],
                   qfs_l[h][:, qsl], start=True, stop=True)
                sc_sb = pb.tile([CH, 2 * CH], bf16, tag="sc_sb")
                nc.vector.tensor_tensor(sc_sb[:, 0:CH], sc_ps[:, 0:CH],
                                        tril, OP.mult)
                if n > CH:
                    nc.vector.tensor_copy(sc_sb[:, CH:n], sc_ps[:, CH:n])
                last = bi == len(HALF_BLOCKS[h]) - 1
                for dd in range(2):
                    mm(r_ps[:, dd, qsl], v_own[:, kc, dd * CH:(dd + 1) * CH],
                       sc_sb[:, 0:n], start=False,
                       stop=(last and dd == 1), skip_group_check=True)

            rt = pb.tile([CH, 2, HSEG], bf16, tag="rt")
            nc.vector.tensor_copy(rt, r_ps)
            if os.environ.get("DEBUG_DUMP") and it == 0:
                drt = consts.tile([CH, 2, HSEG], f32, tag=f"dbg_rt{h}")
                nc.vector.tensor_copy(drt, rt)
                dma(out=t["d_rt"][:, :, h * HSEG:(h + 1) * HSEG], in_=drt)
            sq = pb.tile([CH, 2, HSEG], bf16, tag="sq")
            nc.gpsimd.tensor_tensor(sq, rt, rt, OP.mult)

            st_ps = psB.tile([33, HSEG], f32, tag="st", bufs=2,
                             name=f"st_ps{h}")
            mm(st_ps[0:1, :], oc264, rt[:, 0, :], start=True, stop=False,
               skip_group_check=True)
            mm(st_ps[0:1, :], oc264, rt[:, 1, :], start=False, stop=True,
               skip_group_check=True)
            mm(st_ps[32:33, :], oc264, sq[:, 0, :], start=True, stop=False,
               skip_group_check=True)
            mm(st_ps[32:33, :], oc264, sq[:, 1, :], start=False, stop=False,
               skip_group_check=True)
            mm(st_ps[32:33, :], oc264[0:H, :], sqa[:, hsl], start=False,
               stop=False, skip_group_check=True)
            mm(st_ps[32:33, :], onesr[0:1, 0:1], eps_row[:, 0:HSEG],
               start=False, stop=True, skip_group_check=True)
            msq = pb.tile([1, HSEG], f32, tag="lmsq")
            nc.scalar.activation(msq, st_ps[0:1, :], AF.Square, bias=c264)
            m_b = pb.tile([1, HSEG], bf16, tag="m_b")
            nc.vector.tensor_scalar_add(m_b, st_ps[0:1, :], 1.0 / (D + H))
            rstd = ln_rstd(st_ps[32:33, :], msq, "l", HSEG)
            rbb = psB.tile([CH, HSEG], f32, tag="st", bufs=2,
                           name=f"rbb{h}")
            mm(rbb, onesr, rstd, start=True, stop=True)
            rb_sb = pb.tile([CH, HSEG], bf16, tag="rb_sb")
            nc.scalar.copy(rb_sb, rbb)

            cn = pb.tile([CH, 2, HSEG], bf16, tag="cn")
            for dd in range(2):
                nc.vector.tensor_tensor(cn[:, dd, :], rt[:, dd, :], rb_sb,
                                        OP.mult)
            if os.environ.get("DEBUG_DUMP") and it == 0:
                dcn = consts.tile([CH, 2, HSEG], f32, tag=f"dbg_cn{h}")
                nc.vector.tensor_copy(dcn, cn)
                dma(out=t["d_cn"][:, :, h * HSEG:(h + 1) * HSEG], in_=dcn)
                drs = consts.tile([1, HSEG], f32, tag=f"dbg_rs{h}")
                nc.vector.tensor_copy(drs, rstd)
                dma(out=t["d_rstd"][h:h + 1, 0:HSEG], in_=drs)
            cnam = pb.tile([33, HSEG], bf16, tag="cna")
            nc.vector.memset(cnam, 0.0)
            nc.vector.tensor_tensor(cnam[0:H, :], at[:, hsl], rb_sb[0:H, :],
                                    OP.mult)
            nc.vector.tensor_tensor(cnam[32:33, :], m_b, rstd, OP.mult)

            hh = pb.tile([CH, 4, HSEG], bf16, tag="h")
            for o in range(4):
                h_ps = psB.tile([CH, HSEG], f32, tag="hps", bufs=2,
                                name="h_ps")
                mm(h_ps, w1k(it, 0, o), cn[:, 0, :], start=True, stop=False)
                mm(h_ps, w1k29(it, o), cnam, start=False, stop=False)
                mm(h_ps, w1k(it, 1, o), cn[:, 1, :], start=False, stop=True)
                nc.scalar.activation(hh[:, o, :], h_ps, AF.Silu, scale=GSC,
                                     bias=cF[:, F_B1 + it * 4 + o:
                                             F_B1 + it * 4 + o + 1])
                if os.environ.get("DEBUG_DUMP") and it == 0:
                    dh = consts.tile([CH, HSEG], f32, tag=f"dbg_h{h}{o}")
                    nc.vector.tensor_copy(dh, hh[:, o, :])
                    dma(out=t["d_hh"][:, o, h * HSEG:(h + 1) * HSEG], in_=dh)

            for mh in range(2):
                rf_ps = psB.tile([CH, HSEG], f32, tag="hps", bufs=2,
                                 name="rf_ps")
                for kh in range(4):
                    mm(rf_ps, w2k(it, kh, mh), hh[:, kh, :],
                       start=(kh == 0), stop=(kh == 3))
                nc.scalar.activation(rf[:, mh, hsl], rf_ps, AF.Identity,
                                     bias=cF[:, F_B2 + it * 2 + mh:
                                             F_B2 + it * 2 + mh + 1])
            nc.vector.tensor_tensor(acc_l[h], acc_l[h],
                                    rf[:, :, hsl], OP.add)

            if it < I - 1:
                for mh in range(2):
                    g_ps = psB.tile([CH, HSEG], f32, tag="hps", bufs=2,
                                    name="g_ps")
                    for kh in range(4):
                        rhs = (q[:, kh, hsl] if kh < 2
                               else rf[:, kh - 2, hsl])
                        mm(g_ps, gwk(it, kh, mh), rhs,
                           start=(kh == 0), stop=(kh == 3))
                    nc.scalar.activation(gd[:, mh, hsl], g_ps, AF.Tanh,
                                         bias=cF[:, F_GB + it * 2 + mh:
                                                 F_GB + it * 2 + mh + 1])
        if it < I - 1:
            nc.vector.tensor_tensor(qn, q, gd, OP.add)
        if os.environ.get("DEBUG_DUMP") and it == 0:
            drf = consts.tile([CH, 2, SEG], f32, tag="dbg_rf")
            nc.vector.tensor_copy(drf, rf)
            dma(out=t["d_rf"], in_=drf)
            dq1 = consts.tile([CH, 2, SEG], f32, tag="dbg_q1")
            nc.vector.tensor_copy(dq1, qn)
            dma(out=t["d_q1"], in_=dq1)

    # =========== final LN(acc) @ wog + x (+boe), per token-half ===========
    yt = pb.tile([CH, 2, SEG], f32, tag="yt")
    for h in range(2):
        hsl = slice(h * HSEG, (h + 1) * HSEG)
        sqf = pb.tile([CH, 2, HSEG], bf16, tag="sq")
        nc.scalar.activation(sqf, acc_l[h], AF.Square)
        stf = psB.tile([33, HSEG], f32, tag="st", bufs=2, name=f"stf{h}")
        mm(stf[0:1, :], oc256, acc_l[h][:, 0, :], start=True, stop=False,
           skip_group_check=True)
        mm(stf[0:1, :], oc256, acc_l[h][:, 1, :], start=False, stop=True,
           skip_group_check=True)
        mm(stf[32:33, :], oc256, sqf[:, 0, :], start=True, stop=False,
           skip_group_check=True)
        mm(stf[32:33, :], oc256, sqf[:, 1, :], start=False, stop=False,
           skip_group_check=True)
        mm(stf[32:33, :], onesr[0:1, 0:1], eps_row[:, 0:HSEG],
           start=False, stop=True, skip_group_check=True)
        msqf = pb.tile([1, HSEG], f32, tag="lmsq")
        nc.scalar.activation(msqf, stf[0:1, :], AF.Square)
        m_f = pb.tile([1, HSEG], bf16, tag="m_b")
        nc.vector.tensor_copy(m_f, stf[0:1, :])
        rstdf = ln_rstd(stf[32:33, :], msqf, "l", HSEG)
        mrf = pb.tile([1, HSEG], bf16, tag="mr")
        nc.vector.tensor_tensor(mrf, m_f, rstdf, OP.mult)
        rbf = psB.tile([CH, HSEG], f32, tag="st", bufs=2, name=f"rbf{h}")
        mm(rbf, onesr, rstdf, start=True, stop=True)
        rbf_sb = pb.tile([CH, HSEG], bf16, tag="rb_sb")
        nc.scalar.copy(rbf_sb, rbf)
        cnf = pb.tile([CH, 2, HSEG], bf16, tag="cn")
        for dd in range(2):
            nc.vector.tensor_tensor(cnf[:, dd, :], acc_l[h][:, dd, :],
                                    rbf_sb, OP.mult)
        for mh in range(2):
            o_ps = psB.tile([CH, HSEG], f32, tag="hps", bufs=2, name="o_ps")
            mm(o_ps, wog(0, mh), cnf[:, 0, :], start=True, stop=False)
            mm(o_ps, wog(1, mh), cnf[:, 1, :], start=False, stop=False)
            mm(o_ps, wogsum(mh), mrf, start=False, stop=True)
            nc.vector.tensor_tensor(yt[:, mh, hsl], o_ps, xfm_sb[:, mh, hsl],
                                    OP.add)
    if not os.environ.get("DEBUG_RT"):
        dma(out=t["y"], in_=yt)

    for pool in (psB, pb, pa, own, consts):
        pool.release()
    lp.__exit__(None, None, None)


def _prep_inputs(inputs):
    """Host-side parameter folding + per-core input maps."""
    f = lambda a: np.asarray(a, dtype=np.float32)
    tobf = lambda a: np.ascontiguousarray(
        np.asarray(a, dtype=np.float32)).astype(ml_dtypes.bfloat16)
    x = f(inputs["x"])
    pe_w, pe_b = f(inputs["pe_w"]), f(inputs["pe_b"])
    tv_w, tv_b = f(inputs["tv_w"]), f(inputs["tv_b"])
    mq_w, mq_b = f(inputs["mq_w"]), f(inputs["mq_b"])
    ln_g, ln_b = f(inputs["ref_ln_g"]), f(inputs["ref_ln_b"])
    w1, b1 = f(inputs["ref_w1"]), f(inputs["ref_b1"])
    w2, b2 = f(inputs["ref_w2"]), f(inputs["ref_b2"])
    gw, gb = f(inputs["gate_w"]), f(inputs["gate_b"])
    og, ob = f(inputs["out_ln_g"]), f(inputs["out_ln_b"])
    ow, obias = f(inputs["out_w"]), f(inputs["out_b"])

    w1g = ln_g[:, :, None] * w1                       # (I, 264, 512)
    b1e = b1 + np.einsum("if,ifo->io", ln_b, w1)      # (I, 512)
    w2s = w2 / GSC
    wogm = og[:, None] * ow                           # (256, 256)
    boe = obias + ob @ ow

    # ---- blobA ----
    blobA = np.zeros((CH, A_COLS), np.float32)
    tvpe = np.concatenate([tv_w, pe_w], axis=1)       # (256, 288)
    blobA[:, A_TVPE:A_TVPE + 288] = tvpe[0:128]
    blobA[:, A_TVPE + 288:A_TVPE + 576] = tvpe[128:256]
    blobA[:, A_ID:A_ID + CH] = np.eye(CH)
    blobA[:, A_TRIL:A_TRIL + CH] = np.triu(np.ones((CH, CH)))
    blobA[:, A_OC264] = 1.0 / (D + H)
    blobA[:, A_OC256] = 1.0 / D
    blobA[:, A_ONEC] = 1.0
    blobA[0, A_ONER:A_ONER + 4 * CH] = 1.0
    blobA[0, A_PEBR:A_PEBR + P] = pe_b
    blobA[0, A_MQBH:A_MQBH + H] = mq_b

    # ---- blobB ----
    blobB = np.zeros((CH, B_COLS), np.float32)
    for kh in range(2):
        blobB[:, B_PEW + kh * P:B_PEW + (kh + 1) * P] = \
            pe_w[kh * CH:(kh + 1) * CH]
        blobB[:, B_MQW + kh * H:B_MQW + (kh + 1) * H] = \
            mq_w[kh * CH:(kh + 1) * CH]
        for mh in range(2):
            blobB[:, B_WOG + (kh * 2 + mh) * CH:
                  B_WOG + (kh * 2 + mh + 1) * CH] = \
                wogm[kh * CH:(kh + 1) * CH, mh * CH:(mh + 1) * CH]
    for it in range(I):
        for kh in range(2):
            blobB[:, B_W1K + it * 1024 + kh * 512:
                  B_W1K + it * 1024 + (kh + 1) * 512] = \
                w1g[it, kh * CH:(kh + 1) * CH, :]
        for kh in range(4):
            for mh in range(2):
                blobB[:, B_W2K + it * 1024 + (kh * 2 + mh) * CH:
                      B_W2K + it * 1024 + (kh * 2 + mh + 1) * CH] = \
                    w2s[it, kh * CH:(kh + 1) * CH, mh * CH:(mh + 1) * CH]
        blobB[0:H, B_W1K2 + it * 512:B_W1K2 + (it + 1) * 512] = \
            w1g[it, D:D + H, :]
        blobB[32, B_W1K2 + it * 512:B_W1K2 + (it + 1) * 512] = \
            -w1g[it].sum(axis=0)
    for it in range(2):
        for kh in range(4):
            for mh in range(2):
                blobB[:, B_GWK + it * 1024 + (kh * 2 + mh) * CH:
                      B_GWK + it * 1024 + (kh * 2 + mh + 1) * CH] = \
                    gw[it, kh * CH:(kh + 1) * CH, mh * CH:(mh + 1) * CH]
    blobB[0, B_WOGSUM:B_WOGSUM + D] = -wogm.sum(axis=0)

    # ---- blobF (fp32) ----
    blobF = np.zeros((CH, F_COLS), np.float32)
    blobF[0:P, F_PEB] = pe_b
    blobF[0:H, F_MQB] = 0.5 * mq_b
    for it in range(I):
        blobF[:, F_B1 + it * 4:F_B1 + (it + 1) * 4] = \
            (GSC * b1e[it]).reshape(4, CH).T
        blobF[:, F_B2 + it * 2:F_B2 + (it + 1) * 2] = \
            b2[it].reshape(2, CH).T
    for it in range(2):
        blobF[:, F_GB + it * 2:F_GB + (it + 1) * 2] = \
            gb[it].reshape(2, CH).T
    blobF[:, F_TRIL:F_TRIL + CH] = np.triu(np.ones((CH, CH)))

    blobA = tobf(blobA)
    blobB = tobf(blobB)
    tvb = np.ascontiguousarray(tv_b[None, :])

    in_maps = []
    for core in range(NCORES):
        b, pos = divmod(core, NCORES // B)
        s0 = pos * SEG
        xb = x[b]                                     # (L, D)
        xp = np.zeros((L, D), np.float32)
        xp[0:s0] = xb[0:s0]
        xp[NPREF * CH:NSLOT * CH] = xb[s0:s0 + SEG]
        xp_fm = xp.T                                  # (D, 2048)
        blobC = np.zeros((CH, C_COLS), np.float32)
        blobC[:, C_X:C_X + 2048] = xp_fm[0:CH]
        blobC[:, C_X + 2048:C_X + 4096] = xp_fm[CH:2 * CH]
        gl = np.arange(s0, s0 + SEG, dtype=np.float64)
        iv = (1.0 / (np.sqrt(gl + 1.0) * math.sqrt(P))).astype(np.float32)
        blobC[0:2 * P, C_INVN:C_INVN + SEG] = iv[None, :]
        km = np.zeros(NSLOT, np.float32)
        km[0:4 * pos] = 1.0
        km[NPREF:] = 1.0
        blobC[:, C_KM:C_KM + NSLOT] = km[None, :]
        x_fm = np.zeros((CH, 2, SEG), np.float32)
        xo = xb[s0:s0 + SEG] + boe[None, :]           # (512, 256)
        x_fm[:, 0, :] = xo.T[0:CH]
        x_fm[:, 1, :] = xo.T[CH:2 * CH]
        m = {"blobA_" + _SALT: blobA, "blobB": blobB, "blobF": blobF,
             "tvb": tvb, "x_fm": np.ascontiguousarray(x_fm),
             "blobC": tobf(blobC)}
        in_maps.append(m)
    return in_maps


def kernel(**inputs):
    from concourse.bass_utils import run_bass_kernel_spmd

    if "nc" not in _CACHE:
        _CACHE["nc"] = _build_program()
    nc = _CACHE["nc"]
    in_maps = _prep_inputs(inputs)
    res = run_bass_kernel_spmd(nc, in_maps, core_ids=list(range(NCORES)))
    out = np.empty((B, L, D), dtype=np.float32)
    for core in range(NCORES):
        b, pos = divmod(core, NCORES // B)
        s0 = pos * SEG
        y = np.asarray(res.results[core]["y"])        # (128, 2, 512)
        out[b, s0:s0 + SEG, :] = y.transpose(1, 0, 2).reshape(D, SEG).T
    return out


def gather(res):
    out = np.empty((B, L, D), dtype=np.float32)
    for core in range(NCORES):
        b, pos = divmod(core, NCORES // B)
        s0 = pos * SEG
        y = np.asarray(res.results[core]["y"])
        out[b, s0:s0 + SEG, :] = y.transpose(1, 0, 2).reshape(D, SEG).T
    return out
